# revision 14
# baseline (speedup 1.0000x reference)
"""Trainium2 Bass kernel for nn_Enhancement_77309412162.

Math reduction (from the reference):
  theta[b,n] = sum_c x[b,c,n]*theta_w[c] + theta_b        (per-sample matvec)
  g[b,n]     = sum_c x[b,c,n]*g_w[c] + g_b
  d_i        = <phi_i, g> with phi_i[n] = sum_c xi[c,n]*phi_w[c] + phi_b
  The (N,N) affinity matrices are rank-1, so
  y[b,n] = s_b * theta[b,n],  s_b = (b/N)*(a_c*d1 + (1-a_c)*d2)
  BN over (B,H,W) reduces to the global scalar mean/var of t = s_b*theta:
  out[b,c,n] = x[b,c,n] + alpha[c]*(t[b,n]-mu) + bn_b[c]
  with alpha[c] = bn_w[c]*W_w[c]/sqrt(W_w[c]^2*var_t + 1e-5).

Sharding: batch-parallel, one sample per core (B=8, 8 cores). The only
cross-core data is an allreduce of [sum(t), sum(t^2)] (8 bytes/core).

v3 design (driven by the v1/v2 NTFF traces):
- ncfw facts measured on this runtime: a fixed init BARRIER on the
  collective stream (ends 62-71us), then the first collective starts
  ~11.3us after max(barrier end, trigger).  If any core triggers AFTER
  its barrier end, cross-core skew serializes into the gather (v2: a
  36us AllGather).  Therefore the ONLY hard pre-trigger requirement is
  that every core triggers BEFORE ~60us; the gather then runs in the
  first post-barrier slot (~73-82 local) at its ~8us mesh floor.
- Loads: HWDGE load throughput caps at ~160GB/s PER RING (stores reach
  ~350), so x/x1/x2 (14.2MB) go j-major across THREE queues (SP ring,
  ACT ring, SWDGE) to reach the ~358GB/s HBM cap (~43us).  SWDGE
  carries the latest-landing x1/x2 tiles and casts them fp32->bf16 in
  flight.
- d1/d2 are computed as phi_w^T (Xi @ g) + phi_b*sum(g): DVE
  tensor_tensor_reduce (fused multiply+reduce) of each Xi j-tile
  against a g-broadcast, then a per-channel phi_w weighting and one
  tiny PE column-sum.  This removes the 23us phi matmul stream that
  made v2's PE the pre-trigger bottleneck (PE now only carries the
  theta/g projection and the theta/g broadcasts, ~22us with slack).
- theta/g broadcast to 128 partitions via exact fp32 selector matmuls
  into PSUM + ACT copies (no DRAM bounce in the load window).
- Tail after the AG completes: stride-0 readback, ~11-op DVE chain
  (n1 = bn_w*W_w*s prehoisted), apply in quarter tiles (ACT scale/bias
  + DVE residual add), stores in HALF tiles alternating across both
  HWDGE rings (v2's quarter-tile stores were descriptor-bound).
"""

import os
import numpy as np

B, C, H, W = 8, 512, 48, 48
N = H * W            # 2304
P = 128
J = C // P           # 4 channel chunks
NCHUNKS = [(0, 512), (512, 512), (1024, 512), (1536, 512), (2048, 256)]
QUART = N // 4       # 576
HALF = N // 2        # 1152
NCORES = 8
BN_COUNT = float(B * N)

_cache = {}


def _build_nc():
    import concourse.bass as bass
    import concourse.bacc as bacc
    import concourse.tile as tile
    from concourse import mybir
    from contextlib import ExitStack

    f32 = mybir.dt.float32
    bf16 = mybir.dt.bfloat16
    Alu = mybir.AluOpType
    Act = mybir.ActivationFunctionType
    AxX = mybir.AxisListType.X

    nc = bacc.Bacc("TRN2", target_bir_lowering=False, debug=False,
                   enable_asserts=False, num_devices=NCORES)

    x_d = nc.dram_tensor("x", [C, N], f32, kind="ExternalInput").ap()
    x1_d = nc.dram_tensor("x1", [C, N], f32, kind="ExternalInput").ap()
    x2_d = nc.dram_tensor("x2", [C, N], f32, kind="ExternalInput").ap()
    thw_d = nc.dram_tensor("theta_w", [C], f32, kind="ExternalInput").ap()
    gw_d = nc.dram_tensor("g_w", [C], f32, kind="ExternalInput").ap()
    phw_d = nc.dram_tensor("phi_w", [C], f32, kind="ExternalInput").ap()
    thb_d = nc.dram_tensor("theta_b", [1], f32, kind="ExternalInput").ap()
    gb_d = nc.dram_tensor("g_b", [1], f32, kind="ExternalInput").ap()
    phb_d = nc.dram_tensor("phi_b", [1], f32, kind="ExternalInput").ap()
    ww_d = nc.dram_tensor("W_w", [C], f32, kind="ExternalInput").ap()
    bnw_d = nc.dram_tensor("bn_w", [C], f32, kind="ExternalInput").ap()
    bnb_d = nc.dram_tensor("bn_b", [C], f32, kind="ExternalInput").ap()
    a_d = nc.dram_tensor("a", [1], f32, kind="ExternalInput").ap()
    b_d = nc.dram_tensor("b", [1], f32, kind="ExternalInput").ap()
    out_d = nc.dram_tensor("out", [C, N], f32, kind="ExternalOutput").ap()

    def bcast1(ap_d):
        return bass.AP(tensor=ap_d.tensor, offset=ap_d.offset,
                       ap=[[0, P], [1, 1]])

    with tile.TileContext(nc) as tc, ExitStack() as ctx:
        singles = ctx.enter_context(tc.tile_pool(name="singles", bufs=1))
        tmps = ctx.enter_context(tc.tile_pool(name="tmps", bufs=3))
        scr2 = ctx.enter_context(tc.tile_pool(name="scr2", bufs=2))
        vdump = ctx.enter_context(tc.tile_pool(name="vdump", bufs=2))
        psproj = ctx.enter_context(tc.tile_pool(name="psproj", bufs=1, space="PSUM"))
        psbc = ctx.enter_context(tc.tile_pool(name="psbc", bufs=1, space="PSUM"))
        psr = ctx.enter_context(tc.tile_pool(name="psr", bufs=1, space="PSUM"))
        dram = ctx.enter_context(tc.tile_pool(name="dram", bufs=1, space="DRAM"))

        if int(os.environ.get("KERNEL_CC_WARM", "0")):
            warm_in = dram.tile([1, 2], f32, name="warm_in")
            warm_out = dram.tile([1, 2 * NCORES], f32, name="warm_out")
            nc.gpsimd.collective_compute(
                "AllGather", Alu.bypass,
                replica_groups=[list(range(NCORES))],
                ins=[warm_in.opt()], outs=[warm_out.opt()],
            )

        # ---- tiles ----
        x_tiles = [singles.tile([P, N], f32, name=f"xt{j}") for j in range(J)]
        xbt = [singles.tile([P, N], bf16, name=f"xbt{j}") for j in range(J)]
        # x1: j0/j1/j2 fp32 (HWDGE), j3 bf16 (SWDGE cast-in-flight)
        # x2: j0/j1 fp32 (HWDGE), j2/j3 bf16 (SWDGE)
        x1_f = {j: singles.tile([P, N], f32, name=f"x1f{j}") for j in (0, 1, 2)}
        x1_b = {3: singles.tile([P, N], bf16, name="x1b3")}
        x2_f = {j: singles.tile([P, N], f32, name=f"x2f{j}") for j in (0, 1)}
        x2_b = {j: singles.tile([P, N], bf16, name=f"x2b{j}") for j in (2, 3)}

        # ---- SWDGE (Q7) stream: tiny weight casts, then the 3 bf16 bulk
        # tiles + x j1, then late-needed constants; Q7 then sits idle until
        # the 8-byte cc_in bounce + collective trigger ----
        wxt = singles.tile([P, J, 2], bf16, name="wxt")  # [theta_w | g_w]
        nc.gpsimd.dma_start(out=wxt[:, :, 0],
                            in_=thw_d.rearrange("(j p) -> p j", p=P))
        nc.gpsimd.dma_start(out=wxt[:, :, 1],
                            in_=gw_d.rearrange("(j p) -> p j", p=P))
        nc.gpsimd.dma_start(out=x_tiles[1], in_=x_d[P:2 * P, :])
        nc.gpsimd.dma_start(out=x1_b[3], in_=x1_d[3 * P:4 * P, :])
        nc.gpsimd.dma_start(out=x2_b[2], in_=x2_d[2 * P:3 * P, :])
        nc.gpsimd.dma_start(out=x2_b[3], in_=x2_d[3 * P:4 * P, :])
        ww = singles.tile([P, J], f32, name="ww")
        nc.gpsimd.dma_start(out=ww, in_=ww_d.rearrange("(j p) -> p j", p=P))
        bnw = singles.tile([P, J], f32, name="bnw")
        nc.gpsimd.dma_start(out=bnw, in_=bnw_d.rearrange("(j p) -> p j", p=P))
        bnb = singles.tile([P, J], f32, name="bnb")
        nc.gpsimd.dma_start(out=bnb, in_=bnb_d.rearrange("(j p) -> p j", p=P))
        pwpj = singles.tile([P, J], f32, name="pwpj")
        nc.gpsimd.dma_start(out=pwpj, in_=phw_d.rearrange("(j p) -> p j", p=P))

        # ---- SP (sync) ring: 2 small consts then 4 bulk fp32 tiles ----
        thgb = singles.tile([2, 1], f32, name="thgb")   # row0 theta_b, row1 g_b
        nc.sync.dma_start(out=thgb[0:1, :], in_=thb_d[None, :])
        nc.sync.dma_start(out=thgb[1:2, :], in_=gb_d[None, :])
        phb128 = singles.tile([P, 1], f32, name="phb128")
        nc.sync.dma_start(out=phb128, in_=bcast1(phb_d))
        nc.sync.dma_start(out=x_tiles[0], in_=x_d[0:P, :])
        nc.sync.dma_start(out=x1_f[0], in_=x1_d[0:P, :])
        nc.sync.dma_start(out=x2_f[0], in_=x2_d[0:P, :])
        nc.sync.dma_start(out=x1_f[2], in_=x1_d[2 * P:3 * P, :])

        # ---- ACT (scalar) ring: 2 small consts then 4 bulk fp32 tiles ----
        av128 = singles.tile([P, 1], f32, name="av128")
        nc.scalar.dma_start(out=av128, in_=bcast1(a_d))
        bv128 = singles.tile([P, 1], f32, name="bv128")
        nc.scalar.dma_start(out=bv128, in_=bcast1(b_d))
        nc.scalar.dma_start(out=x_tiles[2], in_=x_d[2 * P:3 * P, :])
        nc.scalar.dma_start(out=x_tiles[3], in_=x_d[3 * P:4 * P, :])
        nc.scalar.dma_start(out=x1_f[1], in_=x1_d[P:2 * P, :])
        nc.scalar.dma_start(out=x2_f[1], in_=x2_d[P:2 * P, :])

        # bf16 copies of x for the single-pass theta/g projection
        for j in range(J):
            nc.vector.tensor_copy(xbt[j], x_tiles[j])

        # selector lhsTs (exact 0/1 fp32): sel0 broadcasts partition-0
        # values to all 128 partitions, sel1 broadcasts partition-1 values
        sel0 = singles.tile([2, P], f32, name="sel0")
        nc.vector.memset(sel0, 0.0)
        nc.vector.memset(sel0[0:1, :], 1.0)
        sel1 = singles.tile([2, P], f32, name="sel1")
        nc.vector.memset(sel1, 1.0)
        nc.vector.tensor_sub(sel1, sel1, sel0)


        # partials on named partitions: col0 row0=A=sum(theta) row1=Cg=sum(g),
        # col1 row0=B=sum(theta^2)
        PT = singles.tile([2, 2], f32, name="PT")
        upair = singles.tile([P, 2], f32, name="upair")
        nc.vector.memset(upair, 0.0)

        # warm the sqrt ACT table so the post-collective sqrt is cheap
        sqwarm = singles.tile([P, 1], f32, name="sqwarm")
        nc.scalar.activation(out=sqwarm, in_=av128, func=Act.Sqrt)

        # prehoisted per-channel products (collective-independent)
        ww2 = singles.tile([P, J], f32, name="ww2")
        nc.vector.tensor_mul(ww2, ww, ww)
        alw = singles.tile([P, J], f32, name="alw")
        nc.vector.tensor_mul(alw, bnw, ww)
        ac = singles.tile([P, 1], f32, name="ac")
        nc.vector.tensor_scalar(ac, av128, 0.0, 1.0, op0=Alu.max, op1=Alu.min)
        c1 = singles.tile([P, 1], f32, name="c1")
        nc.vector.tensor_mul(c1, ac, bv128)
        nc.vector.tensor_scalar_mul(c1, c1, 1.0 / float(N))
        c2 = singles.tile([P, 1], f32, name="c2")
        nc.vector.tensor_scalar(c2, ac, 1.0, None, op0=Alu.subtract)
        nc.vector.tensor_mul(c2, c2, bv128)
        nc.vector.tensor_scalar_mul(c2, c2, -1.0 / float(N))

        # ---- theta/g projection (bf16 single-pass PE, chunk-outer) ----
        thg = singles.tile([2, N], f32, name="thg")     # row0 theta, row1 g
        for (n0, nsz) in NCHUNKS:
            ps = psproj.tile([2, 512], f32, name="ps")
            for j in range(J):
                nc.tensor.matmul(ps[:, :nsz], lhsT=wxt[:, j, :],
                                 rhs=xbt[j][:, n0:n0 + nsz],
                                 start=(j == 0), stop=(j == J - 1))
            nc.scalar.activation(out=thg[:, n0:n0 + nsz], in_=ps[:, :nsz],
                                 func=Act.Identity, bias=thgb, scale=1.0)

        # g broadcast to all 128 partitions (fp32 + bf16 flavors for the
        # fused dots), then theta broadcast (fp32, feeds the apply phase)
        gbc = singles.tile([P, N], f32, name="gbc")
        gbc_b = singles.tile([P, N], bf16, name="gbc_b")
        theta_bc = singles.tile([P, N], f32, name="theta_bc")
        for (n0, nsz) in NCHUNKS:
            pg = psbc.tile([P, 512], f32, name="pg")
            nc.tensor.matmul(pg[:, :nsz], lhsT=sel1, rhs=thg[:, n0:n0 + nsz],
                             start=True, stop=True)
            nc.scalar.activation(out=gbc[:, n0:n0 + nsz], in_=pg[:, :nsz],
                                 func=Act.Identity)
            nc.scalar.activation(out=gbc_b[:, n0:n0 + nsz], in_=pg[:, :nsz],
                                 func=Act.Identity)
        for (n0, nsz) in NCHUNKS:
            pb = psbc.tile([P, 512], f32, name="pb")
            nc.tensor.matmul(pb[:, :nsz], lhsT=sel0, rhs=thg[:, n0:n0 + nsz],
                             start=True, stop=True)
            nc.scalar.activation(out=theta_bc[:, n0:n0 + nsz],
                                 in_=pb[:, :nsz], func=Act.Identity)

        # A = sum(theta) & Cg = sum(g) (accum rows 0/1), B = sum(theta^2)
        sq_scr = scr2.tile([2, N], f32, name="sq_scr")
        nc.scalar.activation(out=sq_scr, in_=thg, func=Act.Identity,
                             accum_out=PT[:, 0:1])
        sq_scr2 = scr2.tile([2, N], f32, name="sq_scr2")
        nc.scalar.activation(out=sq_scr2, in_=thg, func=Act.Square,
                             accum_out=PT[:, 1:2])

        # ---- d-dots: v[p,j] = sum_n xi[128j+p, n] * g[n] via fused DVE
        # multiply+reduce, ordered so each op chases its tile's DMA ----
        dkv1 = singles.tile([P, J], f32, name="dkv1")
        dkv2 = singles.tile([P, J], f32, name="dkv2")

        def vdot(xt, gb, dkv, j):
            # DVE elementwise product, then ACT identity pass whose
            # accumulator produces the per-partition row sum (both are
            # v1-proven constructs; fused tensor_tensor_reduce dies at
            # runtime on this stack despite passing MultiCoreSim)
            dump = vdump.tile([P, N], bf16, name="dump")
            nc.vector.tensor_mul(dump, xt, gb)
            dump2 = vdump.tile([P, N], bf16, name="dump2")
            nc.scalar.activation(out=dump2, in_=dump, func=Act.Identity,
                                 accum_out=dkv[:, j:j + 1])

        vdot(x1_b[3], gbc_b, dkv1, 3)
        vdot(x1_f[0], gbc, dkv1, 0)
        vdot(x1_f[1], gbc, dkv1, 1)
        vdot(x2_b[2], gbc_b, dkv2, 2)
        vdot(x2_f[0], gbc, dkv2, 0)
        vdot(x2_f[1], gbc, dkv2, 1)
        vdot(x2_b[3], gbc_b, dkv2, 3)
        vdot(x1_f[2], gbc, dkv1, 2)

        # weight by phi_w and reduce: dsum[p, i] = sum_j phw[128j+p]*v_i[p,j]
        dsum = singles.tile([P, 2], f32, name="dsum")
        mm1 = singles.tile([P, J], f32, name="mm1")
        nc.vector.tensor_mul(mm1, dkv1, pwpj)
        nc.vector.tensor_reduce(dsum[:, 0:1], mm1, axis=AxX, op=Alu.add)
        mm2 = singles.tile([P, J], f32, name="mm2")
        nc.vector.tensor_mul(mm2, dkv2, pwpj)
        nc.vector.tensor_reduce(dsum[:, 1:2], mm2, axis=AxX, op=Alu.add)

        # broadcast scalars to all partitions via PSUM:
        # pr col0-1 = A,B (partition-0 values); col2 = Cg (partition-1);
        # prd = all-ones matmul that column-sums dsum over the partition
        # axis and broadcasts the result to all 128 partitions in one op
        pr = psr.tile([P, 3], f32, name="pr")
        nc.tensor.matmul(pr[:, 0:2], lhsT=sel0, rhs=PT[:, 0:2],
                         start=True, stop=True)
        nc.tensor.matmul(pr[:, 2:3], lhsT=sel1, rhs=PT[:, 0:1],
                         start=True, stop=True)
        onesPP = singles.tile([P, P], f32, name="onesPP")
        nc.vector.memset(onesPP, 1.0)
        prd = psr.tile([P, 2], f32, name="prd")
        nc.tensor.matmul(prd, lhsT=onesPP, rhs=dsum, start=True, stop=True)
        A_ = pr[:, 0:1]
        B_ = pr[:, 1:2]

        # d_i = d_i_raw + phi_b*Cg;  s = c1*d1 + c2*d2; u1 = s*A; u2 = s^2*B
        t1 = singles.tile([P, 1], f32, name="t1")
        nc.vector.tensor_mul(t1, phb128, pr[:, 2:3])
        d1f = singles.tile([P, 1], f32, name="d1f")
        nc.vector.tensor_add(d1f, prd[:, 0:1], t1)
        d2f = singles.tile([P, 1], f32, name="d2f")
        nc.vector.tensor_add(d2f, prd[:, 1:2], t1)
        sv = singles.tile([P, 1], f32, name="sv")
        sv2p = singles.tile([P, 1], f32, name="sv2p")
        nc.vector.tensor_mul(sv, c1, d1f)
        nc.vector.tensor_mul(sv2p, c2, d2f)
        nc.vector.tensor_add(sv, sv, sv2p)
        s2v = singles.tile([P, 1], f32, name="s2v")
        nc.vector.tensor_mul(s2v, sv, sv)
        nc.vector.tensor_mul(upair[:, 0:1], sv, A_)
        nc.vector.tensor_mul(upair[:, 1:2], s2v, B_)
        # n1 = bn_w*W_w*s, prehoisted off the post-collective chain
        n1 = singles.tile([P, J], f32, name="n1")
        nc.vector.tensor_scalar(n1, alw, sv, None, op0=Alu.mult)

        # ---- 8-byte-per-core allreduce across the 8 cores (ncfw AG) ----
        cc_in = dram.tile([1, 2], f32, name="cc_in")
        cc_out = dram.tile([1, 2 * NCORES], f32, name="cc_out")
        nc.gpsimd.dma_start(out=cc_in, in_=upair[0:1, :])
        nc.gpsimd.collective_compute(
            "AllGather", Alu.bypass,
            replica_groups=[list(range(NCORES))],
            ins=[cc_in.opt()], outs=[cc_out.opt()],
        )
        bcG = singles.tile([P, 2 * NCORES], f32, name="bcG")
        nc.sync.dma_start(out=bcG, in_=bass.AP(tensor=cc_out.tensor,
                                               offset=cc_out.offset,
                                               ap=[[0, P], [1, 2 * NCORES]]))
        uu = singles.tile([P, 2], f32, name="uu")
        nc.vector.tensor_reduce(uu, bcG.rearrange("p (r i) -> p i r", i=2),
                                axis=AxX, op=Alu.add)

        # global stats -> per-channel scale/bias (column j = channels j*128+p)
        muvar = singles.tile([P, 2], f32, name="muvar")
        nc.vector.tensor_scalar_mul(muvar, uu, 1.0 / BN_COUNT)
        muv = muvar[:, 0:1]
        musq = singles.tile([P, 1], f32, name="musq")
        nc.vector.tensor_mul(musq, muv, muv)
        varv = singles.tile([P, 1], f32, name="varv")
        nc.vector.tensor_sub(varv, muvar[:, 1:2], musq)
        dv = singles.tile([P, J], f32, name="dv")
        nc.vector.tensor_scalar(dv, ww2, varv, 1e-5, op0=Alu.mult, op1=Alu.add)
        nc.scalar.activation(out=dv, in_=dv, func=Act.Sqrt)
        rst = singles.tile([P, J], f32, name="rst")
        nc.vector.reciprocal(rst, dv)
        scale2 = singles.tile([P, J], f32, name="scale2")
        nc.vector.tensor_mul(scale2, n1, rst)
        alpha = singles.tile([P, J], f32, name="alpha")
        nc.vector.tensor_mul(alpha, alw, rst)
        bias2 = singles.tile([P, J], f32, name="bias2")
        nc.vector.tensor_scalar(bias2, alpha, muv, None, op0=Alu.mult)
        nc.vector.tensor_sub(bias2, bnb, bias2)

        # out = x + scale2[c]*theta_bc + bias2[c]; quarter-tile compute for
        # a fast pipeline fill, half-tile stores alternating across BOTH
        # HWDGE rings (quarter stores are descriptor-bound)
        for j in range(J):
            for h in range(2):
                for qq in range(2):
                    q = 2 * h + qq
                    sl = slice(q * QUART, (q + 1) * QUART)
                    tmp = tmps.tile([P, QUART], f32, name="tmp")
                    nc.scalar.activation(out=tmp, in_=theta_bc[:, sl],
                                         func=Act.Identity,
                                         scale=scale2[:, j:j + 1],
                                         bias=bias2[:, j:j + 1])
                    nc.vector.tensor_add(x_tiles[j][:, sl],
                                         x_tiles[j][:, sl], tmp)
                hs = slice(h * HALF, (h + 1) * HALF)
                eng = nc.sync if (j * 2 + h) % 2 == 0 else nc.scalar
                eng.dma_start(out=out_d[j * P:(j + 1) * P, hs],
                              in_=x_tiles[j][:, hs])

    nc.compile()
    return nc


def kernel(**inputs):
    from concourse import bass_utils

    nc = _cache.get("nc")
    if nc is None:
        nc = _build_nc()
        _cache["nc"] = nc

    def f32c(a):
        return np.ascontiguousarray(np.asarray(a, dtype=np.float32))

    xs = f32c(inputs["x"]).reshape(B, C, N)
    x1s = f32c(inputs["x1"]).reshape(B, C, N)
    x2s = f32c(inputs["x2"]).reshape(B, C, N)
    shared = {
        "theta_w": f32c(inputs["theta_w"]),
        "g_w": f32c(inputs["g_w"]),
        "phi_w": f32c(inputs["phi_w"]),
        "theta_b": f32c(inputs["theta_b"]),
        "g_b": f32c(inputs["g_b"]),
        "phi_b": f32c(inputs["phi_b"]),
        "W_w": f32c(inputs["W_w"]),
        "bn_w": f32c(inputs["bn_w"]),
        "bn_b": f32c(inputs["bn_b"]),
        "a": f32c(inputs["a"]),
        "b": f32c(inputs["b"]),
    }
    in_maps = [
        {"x": xs[c], "x1": x1s[c], "x2": x2s[c], **shared}
        for c in range(NCORES)
    ]
    res = bass_utils.run_bass_kernel_spmd(
        nc, in_maps, core_ids=list(range(NCORES)),
        trace=bool(os.environ.get("BASS_TRACE")),
        tmpdir=os.environ.get("KERNEL_TMPDIR") or None,
    )
    _cache["last_results"] = res
    out = np.stack([res.results[c]["out"] for c in range(NCORES)], axis=0)
    return out.reshape(B, C, H, W)


# revision 19
# speedup vs baseline: 1.0500x; 1.0500x over previous
"""Trainium2 Bass kernel for nn_Enhancement_77309412162.

Math reduction (from the reference):
  theta[b,n] = sum_c x[b,c,n]*theta_w[c] + theta_b        (per-sample matvec)
  g[b,n]     = sum_c x[b,c,n]*g_w[c] + g_b
  phi1[b,n]  = sum_c x1[b,c,n]*phi_w[c] + phi_b
  phi2[b,n]  = sum_c x2[b,c,n]*phi_w[c] + phi_b
  The (N,N) affinity matrices are rank-1, so
  y[b,n] = s_b * theta[b,n],  s_b = (b/N)*(a_c*<phi1,g> + (1-a_c)*<phi2,g>)
  wy[b,c,n] = W_w[c]*t[b,n] + W_b[c],  t = s_b*theta_b
  BN over (B,H,W):  mean[c] = W_w[c]*mu + W_b[c],  var[c] = W_w[c]^2*var_t
  where mu/var_t are the global scalar mean/var of t over all (b,n).
  out[b,c,n] = x[b,c,n] + alpha[c]*(t[b,n]-mu) + bn_b[c]
  with alpha[c] = bn_w[c]*W_w[c]/sqrt(W_w[c]^2*var_t + 1e-5).

Sharding: batch-parallel, one sample per core (B=8, 8 cores). The only
cross-core data is an allreduce of [sum(t), sum(t^2)] (8 bytes/core).

Final design (v1 bottlenecks measured from NTFF traces: 69us of fp32
2-pass PE matmuls, and the ncfw AllGather chain dominating the tail):
- All three projections (theta/g and phi1/phi2) run as single-pass bf16
  matmuls on PE: x1/x2 are cast fp32->bf16 in-flight by SWDGE, x is cast
  on DVE (the fp32 originals stay resident for an exact residual add).
  PE busy drops 69us -> ~31us. Output rel err ~1.4e-3 vs the 2e-2 gate.
- d1/d2 = <phi_i, g> are chunked DVE mul+reduce against the duplicated
  [phi;phi] projection rows so they chase the phi chunks; the scalar s
  uses prehoisted c1/c2 = f(a,b) so only 3 ops sit after the dots.
- The 8-byte allreduce must go through the ncfw AllGather: raw remote
  SBUF DMA (tested: the v1 3-round butterfly, a 1-round XOR all-to-all
  on queues 0 and 1, prepare-early and prepare-late variants) always
  dies at execution on this runtime even though MultiCoreSim passes.
  The ncfw stream has a fixed init barrier (ends ~52-65us wall-clock)
  plus ~11us stream gap; a garbage warmup AllGather issued as the first
  Q7 instruction absorbs both, and the real collective completes ~100us.
  Gating the real trigger on warmup completion was tried and is ~10us
  WORSE (per-core completion skew inflates the real gather) - trigger
  as soon as upair is ready instead.
- Tail: per-channel scale/bias ACT runs on half tiles so the first store
  launches right after the stats land; stores go on the otherwise-idle
  SP HWDGE ring so the ACT engine never blocks on store issue.
"""

import os
import numpy as np

B, C, H, W = 8, 512, 48, 48
N = H * W            # 2304
P = 128
J = C // P           # 4 channel chunks
NCHUNKS = [(0, 512), (512, 512), (1024, 512), (1536, 512), (2048, 256)]
NCORES = 8
BN_COUNT = float(B * N)

_cache = {}


def _build_nc():
    import concourse.bass as bass
    import concourse.bacc as bacc
    import concourse.tile as tile
    from concourse import mybir
    from contextlib import ExitStack

    f32 = mybir.dt.float32
    bf16 = mybir.dt.bfloat16
    phi_dt = bf16 if os.environ.get("KERNEL_PHI_DT", "bf16") == "bf16" else f32
    Alu = mybir.AluOpType
    Act = mybir.ActivationFunctionType
    AxX = mybir.AxisListType.X

    cc_mode = os.environ.get("KERNEL_CC", "ncfw")

    nc = bacc.Bacc("TRN2", target_bir_lowering=False, debug=False,
                   enable_asserts=False, num_devices=NCORES)

    x_d = nc.dram_tensor("x", [C, N], f32, kind="ExternalInput").ap()
    x1_d = nc.dram_tensor("x1", [C, N], f32, kind="ExternalInput").ap()
    x2_d = nc.dram_tensor("x2", [C, N], f32, kind="ExternalInput").ap()
    thw_d = nc.dram_tensor("theta_w", [C], f32, kind="ExternalInput").ap()
    gw_d = nc.dram_tensor("g_w", [C], f32, kind="ExternalInput").ap()
    phw_d = nc.dram_tensor("phi_w", [C], f32, kind="ExternalInput").ap()
    thb_d = nc.dram_tensor("theta_b", [1], f32, kind="ExternalInput").ap()
    gb_d = nc.dram_tensor("g_b", [1], f32, kind="ExternalInput").ap()
    phb_d = nc.dram_tensor("phi_b", [1], f32, kind="ExternalInput").ap()
    ww_d = nc.dram_tensor("W_w", [C], f32, kind="ExternalInput").ap()
    bnw_d = nc.dram_tensor("bn_w", [C], f32, kind="ExternalInput").ap()
    bnb_d = nc.dram_tensor("bn_b", [C], f32, kind="ExternalInput").ap()
    a_d = nc.dram_tensor("a", [1], f32, kind="ExternalInput").ap()
    b_d = nc.dram_tensor("b", [1], f32, kind="ExternalInput").ap()
    out_d = nc.dram_tensor("out", [C, N], f32, kind="ExternalOutput").ap()

    with tile.TileContext(nc) as tc, ExitStack() as ctx:
        singles = ctx.enter_context(tc.tile_pool(name="singles", bufs=1))
        tmps = ctx.enter_context(tc.tile_pool(name="tmps", bufs=3))
        scr = ctx.enter_context(tc.tile_pool(name="scr", bufs=2))
        psproj = ctx.enter_context(tc.tile_pool(name="psproj", bufs=4, space="PSUM"))
        psr = ctx.enter_context(tc.tile_pool(name="psr", bufs=1, space="PSUM"))
        dram = ctx.enter_context(tc.tile_pool(name="dram", bufs=1, space="DRAM"))

        # ---- semaphores for the remote exchange; cleared first on Q7 (the
        # hardware does not zero semaphores between runs, and arrivals only
        # happen ~40us in, long after these clears retire) ----
        if cc_mode == "xchg":
            rsem = nc.alloc_semaphore(name="x_rsem")
            lsem = nc.alloc_semaphore(name="x_lsem")
            nc.gpsimd.sem_clear(rsem)
        elif int(os.environ.get("KERNEL_CC_WARM", "1")):
            # optional warm-up collective (measured: the stream's init
            # barrier runs from NEFF start regardless of triggers, and a
            # warmup only ADDS its own ~9us + stream gaps in front of the
            # real collective, so default off)
            warm_in = dram.tile([1, 2], f32, name="warm_in")
            warm_out = dram.tile([1, 2 * NCORES], f32, name="warm_out")
            nc.gpsimd.collective_compute(
                "AllGather", Alu.bypass,
                replica_groups=[list(range(NCORES))],
                ins=[warm_in.opt()], outs=[warm_out.opt()],
            )

        # ---- small constant loads on the ACT HWDGE ring (issued at program
        # start, before any ACT compute, so they never block the engine) ----
        # theta/g weights as bf16 lhsT (SWDGE casts in flight)
        wxt = singles.tile([P, J, 2], phi_dt, name="wxt")  # [theta_w | g_w]
        nc.gpsimd.dma_start(out=wxt[:, :, 0],
                            in_=thw_d.rearrange("(j p) -> p j", p=P))
        nc.gpsimd.dma_start(out=wxt[:, :, 1],
                            in_=gw_d.rearrange("(j p) -> p j", p=P))

        def load_pj(ap_d, nm):
            t = singles.tile([P, J], f32, name=nm)
            nc.scalar.dma_start(out=t, in_=ap_d.rearrange("(j p) -> p j", p=P))
            return t

        ww = load_pj(ww_d, "ww")
        bnw = load_pj(bnw_d, "bnw")
        bnb = load_pj(bnb_d, "bnb")

        thgb = singles.tile([2, 1], f32, name="thgb")   # row0 theta_b, row1 g_b
        nc.scalar.dma_start(out=thgb[0:1, :], in_=thb_d[None, :])
        nc.scalar.dma_start(out=thgb[1:2, :], in_=gb_d[None, :])
        phb2 = singles.tile([2, 1], f32, name="phb2")
        nc.scalar.dma_start(out=phb2, in_=bass.AP(tensor=phb_d.tensor,
                                                  offset=phb_d.offset,
                                                  ap=[[0, 2], [1, 1]]))
        av128 = singles.tile([P, 1], f32, name="av128")
        nc.scalar.dma_start(out=av128, in_=bass.AP(tensor=a_d.tensor,
                                                   offset=a_d.offset,
                                                   ap=[[0, P], [1, 1]]))
        bv128 = singles.tile([P, 1], f32, name="bv128")
        nc.scalar.dma_start(out=bv128, in_=bass.AP(tensor=b_d.tensor,
                                                   offset=b_d.offset,
                                                   ap=[[0, P], [1, 1]]))

        # phi weights as bf16 lhsT, duplicated so the projection lands as
        # [phi;phi] next to [theta;g] for the aligned d-dot muls (SWDGE casts
        # fp32->bf16 in flight)
        wpt = singles.tile([P, J, 2], phi_dt, name="wpt")
        nc.gpsimd.dma_start(out=wpt[:, :, 0],
                            in_=phw_d.rearrange("(j p) -> p j", p=P))
        nc.gpsimd.dma_start(out=wpt[:, :, 1],
                            in_=phw_d.rearrange("(j p) -> p j", p=P))

        # ---- bulk input streams ----
        # x fp32 on the two HWDGE rings (kept resident for the residual);
        # x1/x2 as bf16 via SWDGE cast-on-DMA (queue 0).
        x_tiles = []
        for j in range(J):
            xt = singles.tile([P, N], f32, name=f"xt{j}")
            eng = nc.sync if j % 2 == 0 else nc.scalar
            eng.dma_start(out=xt, in_=x_d[j * P:(j + 1) * P, :])
            x_tiles.append(xt)
        # bf16 copies of x feed the single-pass theta/g matmuls; the fp32
        # originals stay resident for the exact residual add
        xbt = []
        for j in range(J):
            t = singles.tile([P, N], phi_dt, name=f"xbt{j}")
            nc.vector.tensor_copy(t, x_tiles[j])
            xbt.append(t)
        xb1 = []
        for j in range(J):
            t = singles.tile([P, N], phi_dt, name=f"xb1_{j}")
            nc.gpsimd.dma_start(out=t, in_=x1_d[j * P:(j + 1) * P, :])
            xb1.append(t)
        xb2 = []
        last_bulk = None
        for j in range(J):
            t = singles.tile([P, N], phi_dt, name=f"xb2_{j}")
            last_bulk = nc.gpsimd.dma_start(out=t, in_=x2_d[j * P:(j + 1) * P, :])
            xb2.append(t)

        # selector lhsTs: sel0 broadcasts partition-0 values to all 128
        # partitions, sel1 broadcasts partition-1 values
        sel0 = singles.tile([2, P], f32, name="sel0")
        nc.vector.memset(sel0, 0.0)
        nc.vector.memset(sel0[0:1, :], 1.0)
        sel1 = singles.tile([2, P], f32, name="sel1")
        nc.vector.memset(sel1, 1.0)
        nc.vector.tensor_sub(sel1, sel1, sel0)

        # partials, written as (2,.) pairs; partner row is ignored:
        # col0 row0=A=sum(theta) row1=C=sum(g), col1 row0=B=sum(theta^2),
        # col2 row1=d1=<phi1,g>, col3 row1=d2=<phi2,g> (biases folded in)
        PT = singles.tile([2, 4], f32, name="PT")

        # exchange buffers; upair is memset early so the descriptor-prepares
        # (which Tile sees as readers of upair) can be scheduled long before
        # the real values are computed
        upair = singles.tile([P, 2], f32, name="upair")
        nc.vector.memset(upair, 0.0)
        if cc_mode == "xchg":
            # Prepare the 7 peer sends early (descriptor generation only) on
            # SWDGE queue 1; the descriptors read upair at fire time. The
            # critical block keeps them in Q7 program order; upair/allbuf are
            # memset above, so this block schedules during the load phase.
            allbuf = singles.tile([P, 2 * NCORES], f32, name="allbuf")
            nc.vector.memset(allbuf, 0.0)

        # warm the sqrt ACT table set early so the post-exchange sqrt does
        # not pay the ~2.7us table load on the critical tail
        sqwarm = singles.tile([P, 1], f32, name="sqwarm")
        nc.scalar.activation(out=sqwarm, in_=av128, func=Act.Sqrt)

        # prehoisted per-channel products (collective-independent)
        ww2 = singles.tile([P, J], f32, name="ww2")
        nc.vector.tensor_mul(ww2, ww, ww)
        alw = singles.tile([P, J], f32, name="alw")
        nc.vector.tensor_mul(alw, bnw, ww)
        # s = (b/N)*(a_c*d1 + (1-a_c)*d2) = c1*d1 + c2*d2 with c1/c2
        # prehoisted from a,b alone so the post-dot chain is 3 ops
        ac = singles.tile([P, 1], f32, name="ac")
        nc.vector.tensor_scalar(ac, av128, 0.0, 1.0, op0=Alu.max, op1=Alu.min)
        c1 = singles.tile([P, 1], f32, name="c1")
        nc.vector.tensor_mul(c1, ac, bv128)
        nc.vector.tensor_scalar_mul(c1, c1, 1.0 / float(N))
        c2 = singles.tile([P, 1], f32, name="c2")
        nc.vector.tensor_scalar(c2, ac, 1.0, None, op0=Alu.subtract)
        nc.vector.tensor_mul(c2, c2, bv128)
        nc.vector.tensor_scalar_mul(c2, c2, -1.0 / float(N))

        # ---- theta/g projection (exact fp32, 2-pass PE) ----
        thg = singles.tile([2, N], f32, name="thg")     # row0 theta, row1 g
        for (n0, nsz) in NCHUNKS:
            ps = psproj.tile([2, 512], f32, name="ps")
            for j in range(J):
                nc.tensor.matmul(ps[:, :nsz], lhsT=wxt[:, j, :],
                                 rhs=xbt[j][:, n0:n0 + nsz],
                                 start=(j == 0), stop=(j == J - 1))
            nc.scalar.activation(out=thg[:, n0:n0 + nsz], in_=ps[:, :nsz],
                                 func=Act.Identity, bias=thgb, scale=1.0)

        # t-broadcast: theta replicated to all 128 partitions via a DRAM
        # bounce + stride-0-partition read (DMA only, hidden under x1/x2
        # loads; keeps PE free)
        theta_bc = singles.tile([P, N], f32, name="theta_bc")
        thg_dram = dram.tile([2, N], f32, name="thg_dram")
        nc.sync.dma_start(out=thg_dram, in_=thg)
        nc.sync.dma_start(out=theta_bc, in_=bass.AP(tensor=thg_dram.tensor,
                                                    offset=thg_dram.offset,
                                                    ap=[[0, P], [1, N]]))

        # A = sum(theta) & C = sum(g) (accum rows 0/1), B = sum(theta^2)
        sq_scr = scr.tile([2, N], f32, name="sq_scr")
        nc.scalar.activation(out=sq_scr, in_=thg, func=Act.Identity,
                             accum_out=PT[:, 0:1])
        nc.scalar.activation(out=sq_scr, in_=thg, func=Act.Square,
                             accum_out=PT[:, 1:2])

        # ---- phi projections (bf16 single-pass PE) + chunked d-dots ----
        def phi_dot(xb, pt_col, nm):
            phi = singles.tile([2, N], f32, name=f"phi{nm}")
            dk = singles.tile([2, len(NCHUNKS)], f32, name=f"dk{nm}")
            for k, (n0, nsz) in enumerate(NCHUNKS):
                ps = psproj.tile([2, 512], f32, name="ps")
                for j in range(J):
                    nc.tensor.matmul(ps[:, :nsz], lhsT=wpt[:, j, :],
                                     rhs=xb[j][:, n0:n0 + nsz],
                                     start=(j == 0), stop=(j == J - 1))
                nc.scalar.activation(out=phi[:, n0:n0 + nsz], in_=ps[:, :nsz],
                                     func=Act.Identity, bias=phb2, scale=1.0)
                ds = scr.tile([2, 512], f32, name="d_scr")
                nc.vector.tensor_mul(ds[:, :nsz], phi[:, n0:n0 + nsz],
                                     thg[:, n0:n0 + nsz])
                nc.vector.tensor_reduce(dk[:, k:k + 1], ds[:, :nsz],
                                        axis=AxX, op=Alu.add)
            nc.vector.tensor_reduce(PT[:, pt_col:pt_col + 1], dk,
                                    axis=AxX, op=Alu.add)

        phi_dot(xb1, 2, "1")
        phi_dot(xb2, 3, "2")

        # broadcast partial rows to all 128 partitions; A/B (row 0, ready
        # once x is projected) go early, d1/d2 (row 1, the late path) in one
        # small matmul; the DVE chain reads the PSUM results directly
        pr = psr.tile([P, 4], f32, name="pr")
        nc.tensor.matmul(pr[:, 0:2], lhsT=sel0, rhs=PT[:, 0:2],
                         start=True, stop=True)
        nc.tensor.matmul(pr[:, 2:4], lhsT=sel1, rhs=PT[:, 2:4],
                         start=True, stop=True)
        A_ = pr[:, 0:1]
        B_ = pr[:, 1:2]
        d1_ = pr[:, 2:3]
        d2_ = pr[:, 3:4]

        # s = c1*d1 + c2*d2; u1 = s*A; u2 = s^2*B  (replicated)
        sv = singles.tile([P, 1], f32, name="sv")
        sv2p = singles.tile([P, 1], f32, name="sv2p")
        nc.vector.tensor_mul(sv, c1, d1_)
        nc.vector.tensor_mul(sv2p, c2, d2_)
        nc.vector.tensor_add(sv, sv, sv2p)
        s2v = singles.tile([P, 1], f32, name="s2v")
        nc.vector.tensor_mul(s2v, sv, sv)
        nc.vector.tensor_mul(upair[:, 0:1], sv, A_)
        nc.vector.tensor_mul(upair[:, 1:2], s2v, B_)

        # ---- 8-byte-per-core allreduce across the 8 cores ----
        uu = singles.tile([P, 2], f32, name="uu")
        if cc_mode == "xchg":
            # own contribution into slot 0 (slot k of receiver r holds data
            # from sender r XOR k, so own data belongs in slot 0)
            nc.vector.tensor_copy(allbuf[:, 0:2], upair)
            upair_snap = singles.tile([P, 2], f32, name="upair_snap")
            with tc.tile_critical():
                # the gpsimd copy gives Tile a tracked read of upair, so the
                # descriptor-prepares + trigger that follow in Q7 program
                # order run only after the final upair values are written;
                # trigger_dma(None) lets Tile insert the descriptor-commit
                # handshake before the tail-pointer bump
                nc.gpsimd.tensor_copy(upair_snap, upair)
                for k in range(1, NCORES):
                    rdests = [None] * NCORES
                    rdests[k] = (0, k)
                    nc.gpsimd.remote_dma_broadcast(
                        out_ap=allbuf[:, 2 * k:2 * k + 2], in_ap=upair[:],
                        remote_sem=rsem, local_sem=lsem,
                        rdests=rdests, queue_num=0)
                nc.gpsimd.trigger_dma(None, queue_num=0)
                nc.vector.wait_ge(rsem, 2 * (NCORES - 1))
                nc.vector.tensor_reduce(
                    uu, allbuf.rearrange("p (s i) -> p i s", i=2),
                    axis=AxX, op=Alu.add)
        else:
            cc_in = dram.tile([1, 2], f32, name="cc_in")
            cc_out = dram.tile([1, 2 * NCORES], f32, name="cc_out")
            nc.gpsimd.dma_start(out=cc_in, in_=upair[0:1, :])
            nc.gpsimd.collective_compute(
                "AllGather", Alu.bypass,
                replica_groups=[list(range(NCORES))],
                ins=[cc_in.opt()], outs=[cc_out.opt()],
            )
            bcG = singles.tile([P, 2 * NCORES], f32, name="bcG")
            nc.sync.dma_start(out=bcG, in_=bass.AP(tensor=cc_out.tensor,
                                                     offset=cc_out.offset,
                                                     ap=[[0, P], [1, 2 * NCORES]]))
            nc.vector.tensor_reduce(uu, bcG.rearrange("p (r i) -> p i r", i=2),
                                    axis=AxX, op=Alu.add)

        # global stats -> per-channel scale/bias (column j = channels j*128+p)
        muvar = singles.tile([P, 2], f32, name="muvar")
        nc.vector.tensor_scalar_mul(muvar, uu, 1.0 / BN_COUNT)
        muv = muvar[:, 0:1]
        musq = singles.tile([P, 1], f32, name="musq")
        nc.vector.tensor_mul(musq, muv, muv)
        varv = singles.tile([P, 1], f32, name="varv")
        nc.vector.tensor_sub(varv, muvar[:, 1:2], musq)
        dv = singles.tile([P, J], f32, name="dv")
        nc.vector.tensor_scalar(dv, ww2, varv, 1e-5, op0=Alu.mult, op1=Alu.add)
        nc.scalar.activation(out=dv, in_=dv, func=Act.Sqrt)
        rst = singles.tile([P, J], f32, name="rst")
        nc.vector.reciprocal(rst, dv)
        alpha = singles.tile([P, J], f32, name="alpha")
        nc.vector.tensor_mul(alpha, alw, rst)
        scale2 = singles.tile([P, J], f32, name="scale2")
        nc.vector.tensor_scalar(scale2, alpha, sv, None, op0=Alu.mult)
        bias2 = singles.tile([P, J], f32, name="bias2")
        nc.vector.tensor_scalar(bias2, alpha, muv, None, op0=Alu.mult)
        nc.vector.tensor_sub(bias2, bnb, bias2)

        # out = x + scale2[c]*theta_bc + bias2[c]; ACT on half tiles so the
        # first store launches early; all stores on the idle SP HWDGE ring
        HALF = N // 2
        for j in range(J):
            for h in range(2):
                sl = slice(h * HALF, (h + 1) * HALF)
                tmp = tmps.tile([P, HALF], f32, name="tmp")
                nc.scalar.activation(out=tmp, in_=theta_bc[:, sl],
                                     func=Act.Identity,
                                     scale=scale2[:, j:j + 1],
                                     bias=bias2[:, j:j + 1])
                nc.vector.tensor_add(x_tiles[j][:, sl], x_tiles[j][:, sl],
                                     tmp)
                nc.sync.dma_start(out=out_d[j * P:(j + 1) * P, sl],
                                  in_=x_tiles[j][:, sl])

    nc.compile()
    return nc


def kernel(**inputs):
    from concourse import bass_utils

    nc = _cache.get("nc")
    if nc is None:
        nc = _build_nc()
        _cache["nc"] = nc

    def f32c(a):
        return np.ascontiguousarray(np.asarray(a, dtype=np.float32))

    xs = f32c(inputs["x"]).reshape(B, C, N)
    x1s = f32c(inputs["x1"]).reshape(B, C, N)
    x2s = f32c(inputs["x2"]).reshape(B, C, N)
    shared = {
        "theta_w": f32c(inputs["theta_w"]),
        "g_w": f32c(inputs["g_w"]),
        "phi_w": f32c(inputs["phi_w"]),
        "theta_b": f32c(inputs["theta_b"]),
        "g_b": f32c(inputs["g_b"]),
        "phi_b": f32c(inputs["phi_b"]),
        "W_w": f32c(inputs["W_w"]),
        "bn_w": f32c(inputs["bn_w"]),
        "bn_b": f32c(inputs["bn_b"]),
        "a": f32c(inputs["a"]),
        "b": f32c(inputs["b"]),
    }
    in_maps = [
        {"x": xs[c], "x1": x1s[c], "x2": x2s[c], **shared}
        for c in range(NCORES)
    ]
    res = bass_utils.run_bass_kernel_spmd(
        nc, in_maps, core_ids=list(range(NCORES)),
        trace=bool(os.environ.get("BASS_TRACE")),
        tmpdir=os.environ.get("KERNEL_TMPDIR") or None,
    )
    _cache["last_results"] = res
    out = np.stack([res.results[c]["out"] for c in range(NCORES)], axis=0)
    return out.reshape(B, C, H, W)


# revision 20
# speedup vs baseline: 1.4688x; 1.3989x over previous
"""Trainium2 Bass kernel for nn_Enhancement_77309412162.

Math reduction (from the reference):
  theta[b,n] = sum_c x[b,c,n]*theta_w[c] + theta_b        (per-sample matvec)
  g[b,n]     = sum_c x[b,c,n]*g_w[c] + g_b
  phi1[b,n]  = sum_c x1[b,c,n]*phi_w[c] + phi_b
  phi2[b,n]  = sum_c x2[b,c,n]*phi_w[c] + phi_b
  The (N,N) affinity matrices are rank-1, so
  y[b,n] = s_b * theta[b,n],  s_b = (b/N)*(a_c*<phi1,g> + (1-a_c)*<phi2,g>)
  wy[b,c,n] = W_w[c]*t[b,n] + W_b[c],  t = s_b*theta_b
  BN over (B,H,W):  mean[c] = W_w[c]*mu + W_b[c],  var[c] = W_w[c]^2*var_t
  where mu/var_t are the global scalar mean/var of t over all (b,n).
  out[b,c,n] = x[b,c,n] + alpha[c]*(t[b,n]-mu) + bn_b[c]
  with alpha[c] = bn_w[c]*W_w[c]/sqrt(W_w[c]^2*var_t + 1e-5).

Sharding: batch-parallel, one sample per core (B=8, 8 cores). The only
cross-core data is an allreduce of [sum(t), sum(t^2)] (8 bytes/core).

Final design (v1 bottlenecks measured from NTFF traces: 69us of fp32
2-pass PE matmuls, and the ncfw AllGather chain dominating the tail):
- All three projections (theta/g and phi1/phi2) run as single-pass bf16
  matmuls on PE: x1/x2 are cast fp32->bf16 in-flight by SWDGE, x is cast
  on DVE (the fp32 originals stay resident for an exact residual add).
  PE busy drops 69us -> ~31us. Output rel err ~1.4e-3 vs the 2e-2 gate.
- d1/d2 = <phi_i, g> are chunked DVE mul+reduce against the duplicated
  [phi;phi] projection rows so they chase the phi chunks; the scalar s
  uses prehoisted c1/c2 = f(a,b) so only 3 ops sit after the dots.
- The 8-byte allreduce must go through the ncfw AllGather: raw remote
  SBUF DMA (tested: the v1 3-round butterfly, a 1-round XOR all-to-all
  on queues 0 and 1, prepare-early and prepare-late variants) always
  dies at execution on this runtime even though MultiCoreSim passes.
  The ncfw stream has a fixed init barrier (ends ~52-65us wall-clock)
  plus ~11us stream gap; a garbage warmup AllGather issued as the first
  Q7 instruction absorbs both, and the real collective completes ~100us.
  Gating the real trigger on warmup completion was tried and is ~10us
  WORSE (per-core completion skew inflates the real gather) - trigger
  as soon as upair is ready instead.
- Tail: per-channel scale/bias ACT runs on half tiles so the first store
  launches right after the stats land; stores go on the otherwise-idle
  SP HWDGE ring so the ACT engine never blocks on store issue.
"""

import os
import numpy as np

B, C, H, W = 8, 512, 48, 48
N = H * W            # 2304
P = 128
J = C // P           # 4 channel chunks
NCHUNKS = [(0, 512), (512, 512), (1024, 512), (1536, 512), (2048, 256)]
NCORES = 8
BN_COUNT = float(B * N)

_cache = {}


def _build_nc():
    import concourse.bass as bass
    import concourse.bacc as bacc
    import concourse.tile as tile
    from concourse import mybir
    from contextlib import ExitStack

    f32 = mybir.dt.float32
    bf16 = mybir.dt.bfloat16
    phi_dt = bf16 if os.environ.get("KERNEL_PHI_DT", "bf16") == "bf16" else f32
    Alu = mybir.AluOpType
    Act = mybir.ActivationFunctionType
    AxX = mybir.AxisListType.X

    cc_mode = os.environ.get("KERNEL_CC", "ncfw")

    nc = bacc.Bacc("TRN2", target_bir_lowering=False, debug=False,
                   enable_asserts=False, num_devices=NCORES)

    x_d = nc.dram_tensor("x", [C, N], f32, kind="ExternalInput").ap()
    x1_d = nc.dram_tensor("x1", [C, N], f32, kind="ExternalInput").ap()
    x2_d = nc.dram_tensor("x2", [C, N], f32, kind="ExternalInput").ap()
    thw_d = nc.dram_tensor("theta_w", [C], f32, kind="ExternalInput").ap()
    gw_d = nc.dram_tensor("g_w", [C], f32, kind="ExternalInput").ap()
    phw_d = nc.dram_tensor("phi_w", [C], f32, kind="ExternalInput").ap()
    thb_d = nc.dram_tensor("theta_b", [1], f32, kind="ExternalInput").ap()
    gb_d = nc.dram_tensor("g_b", [1], f32, kind="ExternalInput").ap()
    phb_d = nc.dram_tensor("phi_b", [1], f32, kind="ExternalInput").ap()
    ww_d = nc.dram_tensor("W_w", [C], f32, kind="ExternalInput").ap()
    bnw_d = nc.dram_tensor("bn_w", [C], f32, kind="ExternalInput").ap()
    bnb_d = nc.dram_tensor("bn_b", [C], f32, kind="ExternalInput").ap()
    a_d = nc.dram_tensor("a", [1], f32, kind="ExternalInput").ap()
    b_d = nc.dram_tensor("b", [1], f32, kind="ExternalInput").ap()
    out_d = nc.dram_tensor("out", [C, N], f32, kind="ExternalOutput").ap()

    with tile.TileContext(nc) as tc, ExitStack() as ctx:
        singles = ctx.enter_context(tc.tile_pool(name="singles", bufs=1))
        tmps = ctx.enter_context(tc.tile_pool(name="tmps", bufs=3))
        scr = ctx.enter_context(tc.tile_pool(name="scr", bufs=2))
        psproj = ctx.enter_context(tc.tile_pool(name="psproj", bufs=4, space="PSUM"))
        psr = ctx.enter_context(tc.tile_pool(name="psr", bufs=1, space="PSUM"))
        dram = ctx.enter_context(tc.tile_pool(name="dram", bufs=1, space="DRAM"))

        # ---- semaphores for the remote exchange; cleared first on Q7 (the
        # hardware does not zero semaphores between runs, and arrivals only
        # happen ~40us in, long after these clears retire) ----
        if cc_mode == "xchg":
            rsem = nc.alloc_semaphore(name="x_rsem")
            lsem = nc.alloc_semaphore(name="x_lsem")
            nc.gpsimd.sem_clear(rsem)
        elif int(os.environ.get("KERNEL_CC_WARM", "1")):
            # optional warm-up collective (measured: the stream's init
            # barrier runs from NEFF start regardless of triggers, and a
            # warmup only ADDS its own ~9us + stream gaps in front of the
            # real collective, so default off)
            warm_in = dram.tile([1, 2], f32, name="warm_in")
            warm_out = dram.tile([1, 2 * NCORES], f32, name="warm_out")
            nc.gpsimd.collective_compute(
                "AllGather", Alu.bypass,
                replica_groups=[list(range(NCORES))],
                ins=[warm_in.opt()], outs=[warm_out.opt()],
            )

        # ---- small constant loads on the ACT HWDGE ring (issued at program
        # start, before any ACT compute, so they never block the engine) ----
        # theta/g weights as bf16 lhsT (SWDGE casts in flight)
        wxt = singles.tile([P, J, 2], phi_dt, name="wxt")  # [theta_w | g_w]
        nc.gpsimd.dma_start(out=wxt[:, :, 0],
                            in_=thw_d.rearrange("(j p) -> p j", p=P))
        nc.gpsimd.dma_start(out=wxt[:, :, 1],
                            in_=gw_d.rearrange("(j p) -> p j", p=P))

        def load_pj(ap_d, nm):
            t = singles.tile([P, J], f32, name=nm)
            nc.scalar.dma_start(out=t, in_=ap_d.rearrange("(j p) -> p j", p=P))
            return t

        ww = load_pj(ww_d, "ww")
        bnw = load_pj(bnw_d, "bnw")
        bnb = load_pj(bnb_d, "bnb")

        thgb = singles.tile([2, 1], f32, name="thgb")   # row0 theta_b, row1 g_b
        nc.scalar.dma_start(out=thgb[0:1, :], in_=thb_d[None, :])
        nc.scalar.dma_start(out=thgb[1:2, :], in_=gb_d[None, :])
        phb2 = singles.tile([2, 1], f32, name="phb2")
        nc.scalar.dma_start(out=phb2, in_=bass.AP(tensor=phb_d.tensor,
                                                  offset=phb_d.offset,
                                                  ap=[[0, 2], [1, 1]]))
        av128 = singles.tile([P, 1], f32, name="av128")
        nc.scalar.dma_start(out=av128, in_=bass.AP(tensor=a_d.tensor,
                                                   offset=a_d.offset,
                                                   ap=[[0, P], [1, 1]]))
        bv128 = singles.tile([P, 1], f32, name="bv128")
        nc.scalar.dma_start(out=bv128, in_=bass.AP(tensor=b_d.tensor,
                                                   offset=b_d.offset,
                                                   ap=[[0, P], [1, 1]]))

        # phi weights as bf16 lhsT, duplicated so the projection lands as
        # [phi;phi] next to [theta;g] for the aligned d-dot muls (SWDGE casts
        # fp32->bf16 in flight)
        wpt = singles.tile([P, J, 2], phi_dt, name="wpt")
        nc.gpsimd.dma_start(out=wpt[:, :, 0],
                            in_=phw_d.rearrange("(j p) -> p j", p=P))
        nc.gpsimd.dma_start(out=wpt[:, :, 1],
                            in_=phw_d.rearrange("(j p) -> p j", p=P))

        # ---- bulk input streams ----
        # x fp32 on the two HWDGE rings (kept resident for the residual);
        # x1/x2 as bf16 via SWDGE cast-on-DMA (queue 0).
        x_tiles = []
        for j in range(J):
            xt = singles.tile([P, N], f32, name=f"xt{j}")
            eng = nc.sync if j % 2 == 0 else nc.scalar
            eng.dma_start(out=xt, in_=x_d[j * P:(j + 1) * P, :])
            x_tiles.append(xt)
        # bf16 copies of x feed the single-pass theta/g matmuls; the fp32
        # originals stay resident for the exact residual add
        xbt = []
        for j in range(J):
            t = singles.tile([P, N], phi_dt, name=f"xbt{j}")
            nc.vector.tensor_copy(t, x_tiles[j])
            xbt.append(t)
        xb1 = []
        for j in range(J):
            t = singles.tile([P, N], phi_dt, name=f"xb1_{j}")
            nc.gpsimd.dma_start(out=t, in_=x1_d[j * P:(j + 1) * P, :])
            xb1.append(t)
        xb2 = []
        last_bulk = None
        for j in range(J):
            t = singles.tile([P, N], phi_dt, name=f"xb2_{j}")
            last_bulk = nc.gpsimd.dma_start(out=t, in_=x2_d[j * P:(j + 1) * P, :])
            xb2.append(t)

        # selector lhsTs: sel0 broadcasts partition-0 values to all 128
        # partitions, sel1 broadcasts partition-1 values
        sel0 = singles.tile([2, P], f32, name="sel0")
        nc.vector.memset(sel0, 0.0)
        nc.vector.memset(sel0[0:1, :], 1.0)
        sel1 = singles.tile([2, P], f32, name="sel1")
        nc.vector.memset(sel1, 1.0)
        nc.vector.tensor_sub(sel1, sel1, sel0)

        # partials, written as (2,.) pairs; partner row is ignored:
        # col0 row0=A=sum(theta) row1=C=sum(g), col1 row0=B=sum(theta^2),
        # col2 row1=d1=<phi1,g>, col3 row1=d2=<phi2,g> (biases folded in)
        PT = singles.tile([2, 4], f32, name="PT")

        # exchange buffers; upair is memset early so the descriptor-prepares
        # (which Tile sees as readers of upair) can be scheduled long before
        # the real values are computed
        upair = singles.tile([P, 2], f32, name="upair")
        nc.vector.memset(upair, 0.0)
        if cc_mode == "xchg":
            # Prepare the 7 peer sends early (descriptor generation only) on
            # SWDGE queue 1; the descriptors read upair at fire time. The
            # critical block keeps them in Q7 program order; upair/allbuf are
            # memset above, so this block schedules during the load phase.
            allbuf = singles.tile([P, 2 * NCORES], f32, name="allbuf")
            nc.vector.memset(allbuf, 0.0)

        # warm the sqrt ACT table set early so the post-exchange sqrt does
        # not pay the ~2.7us table load on the critical tail
        sqwarm = singles.tile([P, 1], f32, name="sqwarm")
        nc.scalar.activation(out=sqwarm, in_=av128, func=Act.Sqrt)

        # prehoisted per-channel products (collective-independent)
        ww2 = singles.tile([P, J], f32, name="ww2")
        nc.vector.tensor_mul(ww2, ww, ww)
        alw = singles.tile([P, J], f32, name="alw")
        nc.vector.tensor_mul(alw, bnw, ww)
        # s = (b/N)*(a_c*d1 + (1-a_c)*d2) = c1*d1 + c2*d2 with c1/c2
        # prehoisted from a,b alone so the post-dot chain is 3 ops
        ac = singles.tile([P, 1], f32, name="ac")
        nc.vector.tensor_scalar(ac, av128, 0.0, 1.0, op0=Alu.max, op1=Alu.min)
        c1 = singles.tile([P, 1], f32, name="c1")
        nc.vector.tensor_mul(c1, ac, bv128)
        nc.vector.tensor_scalar_mul(c1, c1, 1.0 / float(N))
        c2 = singles.tile([P, 1], f32, name="c2")
        nc.vector.tensor_scalar(c2, ac, 1.0, None, op0=Alu.subtract)
        nc.vector.tensor_mul(c2, c2, bv128)
        nc.vector.tensor_scalar_mul(c2, c2, -1.0 / float(N))

        # ---- theta/g projection (exact fp32, 2-pass PE) ----
        thg = singles.tile([2, N], f32, name="thg")     # row0 theta, row1 g
        for (n0, nsz) in NCHUNKS:
            ps = psproj.tile([2, 512], f32, name="ps")
            for j in range(J):
                nc.tensor.matmul(ps[:, :nsz], lhsT=wxt[:, j, :],
                                 rhs=xbt[j][:, n0:n0 + nsz],
                                 start=(j == 0), stop=(j == J - 1))
            nc.scalar.activation(out=thg[:, n0:n0 + nsz], in_=ps[:, :nsz],
                                 func=Act.Identity, bias=thgb, scale=1.0)

        # t-broadcast: theta replicated to all 128 partitions via a DRAM
        # bounce + stride-0-partition read (DMA only, hidden under x1/x2
        # loads; keeps PE free)
        theta_bc = singles.tile([P, N], f32, name="theta_bc")
        thg_dram = dram.tile([2, N], f32, name="thg_dram")
        nc.sync.dma_start(out=thg_dram, in_=thg)
        nc.sync.dma_start(out=theta_bc, in_=bass.AP(tensor=thg_dram.tensor,
                                                    offset=thg_dram.offset,
                                                    ap=[[0, P], [1, N]]))

        # A = sum(theta) & C = sum(g) (accum rows 0/1), B = sum(theta^2)
        sq_scr = scr.tile([2, N], f32, name="sq_scr")
        nc.scalar.activation(out=sq_scr, in_=thg, func=Act.Identity,
                             accum_out=PT[:, 0:1])
        nc.scalar.activation(out=sq_scr, in_=thg, func=Act.Square,
                             accum_out=PT[:, 1:2])

        # ---- phi projections (bf16 single-pass PE) + chunked d-dots ----
        def phi_dot(xb, pt_col, nm):
            phi = singles.tile([2, N], f32, name=f"phi{nm}")
            dk = singles.tile([2, len(NCHUNKS)], f32, name=f"dk{nm}")
            for k, (n0, nsz) in enumerate(NCHUNKS):
                ps = psproj.tile([2, 512], f32, name="ps")
                for j in range(J):
                    nc.tensor.matmul(ps[:, :nsz], lhsT=wpt[:, j, :],
                                     rhs=xb[j][:, n0:n0 + nsz],
                                     start=(j == 0), stop=(j == J - 1))
                nc.scalar.activation(out=phi[:, n0:n0 + nsz], in_=ps[:, :nsz],
                                     func=Act.Identity, bias=phb2, scale=1.0)
                ds = scr.tile([2, 512], f32, name="d_scr")
                nc.vector.tensor_mul(ds[:, :nsz], phi[:, n0:n0 + nsz],
                                     thg[:, n0:n0 + nsz])
                nc.vector.tensor_reduce(dk[:, k:k + 1], ds[:, :nsz],
                                        axis=AxX, op=Alu.add)
            nc.vector.tensor_reduce(PT[:, pt_col:pt_col + 1], dk,
                                    axis=AxX, op=Alu.add)

        phi_dot(xb1, 2, "1")
        phi_dot(xb2, 3, "2")

        # broadcast partial rows to all 128 partitions; A/B (row 0, ready
        # once x is projected) go early, d1/d2 (row 1, the late path) in one
        # small matmul; the DVE chain reads the PSUM results directly
        pr = psr.tile([P, 4], f32, name="pr")
        nc.tensor.matmul(pr[:, 0:2], lhsT=sel0, rhs=PT[:, 0:2],
                         start=True, stop=True)
        nc.tensor.matmul(pr[:, 2:4], lhsT=sel1, rhs=PT[:, 2:4],
                         start=True, stop=True)
        A_ = pr[:, 0:1]
        B_ = pr[:, 1:2]
        d1_ = pr[:, 2:3]
        d2_ = pr[:, 3:4]

        # s = c1*d1 + c2*d2; u1 = s*A; u2 = s^2*B  (replicated)
        sv = singles.tile([P, 1], f32, name="sv")
        sv2p = singles.tile([P, 1], f32, name="sv2p")
        nc.vector.tensor_mul(sv, c1, d1_)
        nc.vector.tensor_mul(sv2p, c2, d2_)
        nc.vector.tensor_add(sv, sv, sv2p)
        s2v = singles.tile([P, 1], f32, name="s2v")
        nc.vector.tensor_mul(s2v, sv, sv)
        nc.vector.tensor_mul(upair[:, 0:1], sv, A_)
        nc.vector.tensor_mul(upair[:, 1:2], s2v, B_)

        # ---- 8-byte-per-core allreduce across the 8 cores ----
        uu = singles.tile([P, 2], f32, name="uu")
        if cc_mode == "xchg":
            # own contribution into slot 0 (slot k of receiver r holds data
            # from sender r XOR k, so own data belongs in slot 0)
            nc.vector.tensor_copy(allbuf[:, 0:2], upair)
            upair_snap = singles.tile([P, 2], f32, name="upair_snap")
            with tc.tile_critical():
                # the gpsimd copy gives Tile a tracked read of upair, so the
                # descriptor-prepares + trigger that follow in Q7 program
                # order run only after the final upair values are written;
                # trigger_dma(None) lets Tile insert the descriptor-commit
                # handshake before the tail-pointer bump
                nc.gpsimd.tensor_copy(upair_snap, upair)
                for k in range(1, NCORES):
                    rdests = [None] * NCORES
                    rdests[k] = (0, k)
                    nc.gpsimd.remote_dma_broadcast(
                        out_ap=allbuf[:, 2 * k:2 * k + 2], in_ap=upair[:],
                        remote_sem=rsem, local_sem=lsem,
                        rdests=rdests, queue_num=0)
                nc.gpsimd.trigger_dma(None, queue_num=0)
                nc.vector.wait_ge(rsem, 2 * (NCORES - 1))
                nc.vector.tensor_reduce(
                    uu, allbuf.rearrange("p (s i) -> p i s", i=2),
                    axis=AxX, op=Alu.add)
        else:
            cc_in = dram.tile([1, 2], f32, name="cc_in")
            cc_out = dram.tile([1, 2 * NCORES], f32, name="cc_out")
            nc.gpsimd.dma_start(out=cc_in, in_=upair[0:1, :])
            nc.gpsimd.collective_compute(
                "AllGather", Alu.bypass,
                replica_groups=[list(range(NCORES))],
                ins=[cc_in.opt()], outs=[cc_out.opt()],
            )
            bcG = singles.tile([P, 2 * NCORES], f32, name="bcG")
            nc.sync.dma_start(out=bcG, in_=bass.AP(tensor=cc_out.tensor,
                                                     offset=cc_out.offset,
                                                     ap=[[0, P], [1, 2 * NCORES]]))
            nc.vector.tensor_reduce(uu, bcG.rearrange("p (r i) -> p i r", i=2),
                                    axis=AxX, op=Alu.add)

        # global stats -> per-channel scale/bias (column j = channels j*128+p)
        muvar = singles.tile([P, 2], f32, name="muvar")
        nc.vector.tensor_scalar_mul(muvar, uu, 1.0 / BN_COUNT)
        muv = muvar[:, 0:1]
        musq = singles.tile([P, 1], f32, name="musq")
        nc.vector.tensor_mul(musq, muv, muv)
        varv = singles.tile([P, 1], f32, name="varv")
        nc.vector.tensor_sub(varv, muvar[:, 1:2], musq)
        dv = singles.tile([P, J], f32, name="dv")
        nc.vector.tensor_scalar(dv, ww2, varv, 1e-5, op0=Alu.mult, op1=Alu.add)
        nc.scalar.activation(out=dv, in_=dv, func=Act.Sqrt)
        rst = singles.tile([P, J], f32, name="rst")
        nc.vector.reciprocal(rst, dv)
        alpha = singles.tile([P, J], f32, name="alpha")
        nc.vector.tensor_mul(alpha, alw, rst)
        scale2 = singles.tile([P, J], f32, name="scale2")
        nc.vector.tensor_scalar(scale2, alpha, sv, None, op0=Alu.mult)
        bias2 = singles.tile([P, J], f32, name="bias2")
        nc.vector.tensor_scalar(bias2, alpha, muv, None, op0=Alu.mult)
        nc.vector.tensor_sub(bias2, bnb, bias2)

        # out = x + scale2[c]*theta_bc + bias2[c]; quarter-tile ACT/DVE so
        # the pipeline fills fast, half-tile stores alternating across BOTH
        # HWDGE rings (one ring caps at ~286GB/s; two reach the ~358 HBM
        # write limit — tail drops ~16.5us -> ~13.5us)
        HALF = N // 2
        QUART = N // 4
        for j in range(J):
            for h in range(2):
                for qq in range(2):
                    q = 2 * h + qq
                    sl = slice(q * QUART, (q + 1) * QUART)
                    tmp = tmps.tile([P, QUART], f32, name="tmp")
                    nc.scalar.activation(out=tmp, in_=theta_bc[:, sl],
                                         func=Act.Identity,
                                         scale=scale2[:, j:j + 1],
                                         bias=bias2[:, j:j + 1])
                    nc.vector.tensor_add(x_tiles[j][:, sl],
                                         x_tiles[j][:, sl], tmp)
                hs = slice(h * HALF, (h + 1) * HALF)
                eng = nc.sync if (j * 2 + h) % 2 == 0 else nc.scalar
                eng.dma_start(out=out_d[j * P:(j + 1) * P, hs],
                              in_=x_tiles[j][:, hs])

    nc.compile()
    return nc


def kernel(**inputs):
    from concourse import bass_utils

    nc = _cache.get("nc")
    if nc is None:
        nc = _build_nc()
        _cache["nc"] = nc

    def f32c(a):
        return np.ascontiguousarray(np.asarray(a, dtype=np.float32))

    xs = f32c(inputs["x"]).reshape(B, C, N)
    x1s = f32c(inputs["x1"]).reshape(B, C, N)
    x2s = f32c(inputs["x2"]).reshape(B, C, N)
    shared = {
        "theta_w": f32c(inputs["theta_w"]),
        "g_w": f32c(inputs["g_w"]),
        "phi_w": f32c(inputs["phi_w"]),
        "theta_b": f32c(inputs["theta_b"]),
        "g_b": f32c(inputs["g_b"]),
        "phi_b": f32c(inputs["phi_b"]),
        "W_w": f32c(inputs["W_w"]),
        "bn_w": f32c(inputs["bn_w"]),
        "bn_b": f32c(inputs["bn_b"]),
        "a": f32c(inputs["a"]),
        "b": f32c(inputs["b"]),
    }
    in_maps = [
        {"x": xs[c], "x1": x1s[c], "x2": x2s[c], **shared}
        for c in range(NCORES)
    ]
    res = bass_utils.run_bass_kernel_spmd(
        nc, in_maps, core_ids=list(range(NCORES)),
        trace=bool(os.environ.get("BASS_TRACE")),
        tmpdir=os.environ.get("KERNEL_TMPDIR") or None,
    )
    _cache["last_results"] = res
    out = np.stack([res.results[c]["out"] for c in range(NCORES)], axis=0)
    return out.reshape(B, C, H, W)
